# revision 27
# baseline (speedup 1.0000x reference)
"""Trainium2 Bass kernel for nn_DeepSetAttentionModel (segment_reduce).

Division of labour (device does the three dense 128x128 phi layers, ~90% of
the model FLOPs; host does O(N*small) prep and reductions):
  * Host assembles the 48-dim token features (sin/cos positional enc, value,
    one-hot measurement) and the first embedding layer h0 = relu(W0.T x + b0)
    (K=48, 11% of FLOPs) -- one [128,48]@[48,N] sgemm per core -- shipping
    h0 per core to HBM as fp8e4, pre-split into two 64-partition k-subtile
    planes [64, 2, N] for the PE's DoubleRow mode.
  * The psi-MLP / segment-mean branch adds a per-segment constant per head to
    the attention logits; segment softmax is invariant to it, so the whole
    psi branch cancels and is dropped.
  * Attention logits are rank-4 per token (z = x @ M1 with M1 folded from
    W_k[:48] . W_q); host computes the segment softmax weights exactly.
  * The 64 demo tokens (1 per segment) go through the phi MLP on host in
    f32; the device stream is exactly 8 segments x 4096 time tokens per
    core -- no padding columns.
  * Device: phi layer 1 runs as fp8 DoubleRow matmuls (2 output cols/cycle,
    host supplies the split operand layout for free); layers 2-3 in bf16.
    Emission is layer-major over half-core blocks (L1 over 2 super-blocks,
    then L2, then L3) so every layer's inputs are drained long before the
    PE reaches them -- the pipeline fills exactly once.  PSUM->SBUF relu
    drains alternate between the Activation and Vector engines on 2-bank
    (1024-col) groups, 4 in flight, slightly ACT-biased to match the
    engines' 1.2 vs 0.96 GHz clocks; this pair of engines is the throughput
    floor of the kernel.  enc [128, 32768] fp8e4 streams back to HBM in
    2048-col pieces, overlapped with compute.  All DMA runs on SP's HWDGE
    queue in consumption order (one queue moves ~330GB/s; the 16 DMA
    engines are shared, so a second queue only steals from the head), as
    2D-contiguous per-plane slices -- 3D strided pieces are ~10x slower.
  * Host: attention-weighted segment sums over enc (O(N*512) f32 BLAS),
    then the tiny rho MLP 512->128->128->128->1 + sigmoid.

Sharding: data-parallel across patients -- 8 whole segments per core,
weights replicated, no collectives.
"""

import numpy as np
import ml_dtypes

import concourse.tile as tile
from concourse import bacc, mybir
from concourse.bass_utils import run_bass_kernel_spmd

F32 = mybir.dt.float32
BF16 = mybir.dt.bfloat16
FP8 = mybir.dt.float8e4
AF = mybir.ActivationFunctionType
ALU = mybir.AluOpType
NPBF16 = ml_dtypes.bfloat16

NCORES = 8
B, T = 64, 4096
SEG = 8                  # segments per core
N = SEG * T              # 32768 device tokens per core
D_IN = 48
HEADS, DOT = 4, 64
N_MOD = 37

SB_SEGS = 2              # segments per super-block
NSB = SEG // SB_SEGS     # 4 super-blocks
SBW = SB_SEGS * T        # 8192 cols per super-block
CH = 512                 # matmul moving width (one PSUM bank of f32)
GRP = 2 * CH             # drain group: 2 banks = 1024 cols
NGRP = SBW // GRP        # 8 drain groups per layer per super-block
ENC_DMA = 2 * GRP        # enc DMA piece: 2048 cols (512KB)

_CACHE = {}


def _build(zero_b):
    nc = bacc.Bacc(
        "TRN2",
        target_bir_lowering=False,
        debug=False,
        enable_asserts=False,
        num_devices=NCORES,
    )

    io = {
        "h0in": nc.dram_tensor("h0in", [64, 2, N], FP8,
                               kind="ExternalInput").ap(),
        "wpack": nc.dram_tensor("wpack", [128, 256], BF16,
                                kind="ExternalInput").ap(),
        "wpack8": nc.dram_tensor("wpack8", [64, 256], FP8,
                                 kind="ExternalInput").ap(),
        "cpack": nc.dram_tensor("cpack", [128, 3], F32,
                                kind="ExternalInput").ap(),
        "enc": nc.dram_tensor("enc", [128, N], FP8, kind="ExternalOutput").ap(),
    }

    with tile.TileContext(nc) as tc:
        _emit(tc, io, zero_b)

    _dedup_ldweights(nc)
    nc.compile()
    return nc


def _ldw_key(inst):
    ap = inst.ins[0]
    return (
        getattr(ap, "memref", None),
        ap.offset,
        tuple(tuple(p) for p in ap.ap),
        str(ap.dtype),
        str(getattr(inst, "tile_position", None)),
        str(getattr(inst, "perf_mode", None)),
        bool(inst.is_transpose or False),
    )


def _dedup_ldweights(nc):
    """Drop InstLdweights that reload the stationary operand already in the
    PE array (identical weights AP, no intervening PE weight writes).  The
    PE keeps weights across matmuls, so the reload is semantically a no-op
    but costs ~90ns and breaks back-to-back matmul fill/drain pipelining.
    Dropped instructions' semaphore waits transfer to the next PE
    instruction."""
    removed = 0
    for fn in nc.m.functions:
        for b in fn.blocks:
            last_key = None
            pending_waits = []
            keep = []
            for inst in b.instructions:
                eng = getattr(inst, "engine", None)
                if isinstance(inst, mybir.InstLdweights):
                    key = _ldw_key(inst)
                    si = inst.sync_info
                    if key == last_key and not (si and si.on_update):
                        if si and si.on_wait:
                            pending_waits.extend(si.on_wait)
                        removed += 1
                        continue
                    last_key = key
                elif isinstance(inst, mybir.InstMatmult):
                    if inst.ldweights:
                        last_key = None
                elif eng == mybir.EngineType.PE and not inst.is_sequencer_only():
                    last_key = None
                if pending_waits and eng == mybir.EngineType.PE:
                    si = inst.sync_info
                    if si is None:
                        inst.sync_info = mybir.SyncInfo(
                            on_wait=list(pending_waits), on_update=[])
                    else:
                        si.on_wait = list(si.on_wait) + pending_waits
                    pending_waits = []
                keep.append(inst)
            assert not pending_waits, "dropped LDW waits with no PE successor"
            b.instructions[:] = keep
    return removed


def _emit(tc, io, zero_b):
    nc = tc.nc
    sync = nc.sync
    act = nc.scalar
    dve = nc.vector
    pe = nc.tensor

    with tc.tile_pool(name="const", bufs=1) as cp:
        wsb8 = cp.tile([64, 2, 128], FP8, tag="wsb8")
        for p8 in range(2):
            sync.dma_start(wsb8[:, p8, :], io["wpack8"][:, p8 * 128:(p8 + 1) * 128])

        # h0 super-block tiles.  One HWDGE queue moves ~330GB/s and each
        # dma_start costs SP ~600ns to issue, so pieces are per-plane 2D
        # slices (3D strided DMA runs ~10x slower) sized to keep arrival
        # ahead of the PE with few issues: sb0 in 256KB halves (both planes
        # of the first 4096 cols land before the first matmul), the rest as
        # whole-plane 512KB transfers, all in consumption order.
        hb = [cp.tile([64, 2, SBW], FP8, tag=f"h0_{sb}", name=f"h0_{sb}")
              for sb in range(NSB)]
        head = 2048
        for p in range(2):
            sync.dma_start(hb[0][:, p, 0:head], io["h0in"][:, p, 0:head])
        for p in range(2):
            sync.dma_start(hb[0][:, p, head:SBW],
                           io["h0in"][:, p, head:SBW])
        wsb = cp.tile([128, 256], BF16, tag="wsb")
        sync.dma_start(wsb, io["wpack"])
        csb = cp.tile([128, 3], F32, tag="csb")
        sync.dma_start(csb, io["cpack"])
        for sb in range(1, NSB):
            for p in range(2):
                sync.dma_start(hb[sb][:, p, :],
                               io["h0in"][:, p, sb * SBW:(sb + 1) * SBW])

        w = [wsb8, wsb[:, 0:128], wsb[:, 128:256]]
        pb = [csb[:, i:i + 1] for i in range(3)]

        # Warm each drain engine during the DMA prologue: the first ACT use
        # pays a ~1.3us activation-table load, the first DVE op pays a
        # similar tensor load.
        wa = cp.tile([1, 1], F32, tag="warma")
        dve.memset(wa, 0.0)
        act.activation(wa, wa, AF.Relu)
        wv = cp.tile([1, 1], F32, tag="warmv")
        dve.memset(wv, 0.0)
        dve.tensor_scalar(wv, wv, 0.0, 0.0, ALU.add, ALU.max)

        # Half-core layer-major schedule: L1 over two super-blocks, then L2,
        # then L3.  Every layer's inputs are fully drained long before the
        # PE reaches them, so there are no layer-transition bubbles -- the
        # pipeline fills exactly once, at the very start.
        with tc.tile_pool(name="hbuf", bufs=2) as hp, \
             tc.tile_pool(name="encb", bufs=2) as ep, \
             tc.tile_pool(name="ps", bufs=4, space="PSUM") as pp:
            for half in range(NSB // 2):
                sbs = (2 * half, 2 * half + 1)
                h1 = {s: hp.tile([128, SBW], BF16, tag="h1", name=f"h1_{s}")
                      for s in sbs}
                h2 = {s: hp.tile([128, SBW], BF16, tag="h2", name=f"h2_{s}")
                      for s in sbs}
                enc = {s: ep.tile([128, SBW], FP8, tag="enc", name=f"enc_{s}")
                       for s in sbs}
                for li in range(3):
                    for sb in sbs:
                        src = (hb[sb], h1[sb], h2[sb])[li]
                        dst = (h1[sb], h2[sb], enc[sb])[li]
                        wk, bk = w[li], pb[li]
                        # ACT (1.2GHz) drains slightly more groups than DVE
                        # (0.96GHz): every 4th pass runs ADADADAA.
                        ppass = li * NSB + sb
                        pat = "ADADADAA" if ppass % 4 == 1 else "ADADADAD"
                        for g in range(NGRP):
                            ps = pp.tile([128, GRP], F32, tag="ps",
                                         name=f"ps{sb}_{li}_{g}")
                            for c in range(GRP // CH):
                                col = g * GRP + c * CH
                                if li == 0:
                                    # fp8 DoubleRow: K=128 as two 64-row
                                    # subtiles; host ships h0/w1 pre-split.
                                    pe.matmul(ps[:, c * CH:(c + 1) * CH], wk,
                                              src[:, :, col:col + CH],
                                              start=True, stop=True,
                                              perf_mode=mybir.MatmulPerfMode
                                              .DoubleRow)
                                else:
                                    pe.matmul(ps[:, c * CH:(c + 1) * CH], wk,
                                              src[:, col:col + CH],
                                              start=True, stop=True)
                            dv = dst[:, g * GRP:(g + 1) * GRP]
                            if pat[g] == "A":
                                if zero_b:
                                    act.activation(dv, ps, AF.Relu)
                                else:
                                    act.activation(dv, ps, AF.Relu, bias=bk)
                            elif zero_b:
                                dve.tensor_scalar_max(dv, ps, 0.0)
                            else:
                                dve.tensor_scalar(dv, ps, bk, 0.0,
                                                  ALU.add, ALU.max)
                            piece = ENC_DMA
                            if li == 2 and (g + 1) % (piece // GRP) == 0:
                                base = (g + 1) * GRP - piece
                                seg0 = sb * SB_SEGS
                                sync.dma_start(
                                    io["enc"][:, seg0 * T + base:
                                              seg0 * T + base + piece],
                                    dst[:, base:base + piece])


def get_nc(zero_b):
    key = ("nc", zero_b)
    if key not in _CACHE:
        _CACHE[key] = _build(zero_b)
    return _CACHE[key]


def _fp8_to_f32(a):
    lut = _CACHE.get("fp8lut")
    if lut is None:
        lut = np.arange(256, dtype=np.uint8).view(ml_dtypes.float8_e4m3) \
            .astype(np.float32)
        _CACHE["fp8lut"] = lut
    return lut[np.asarray(a).view(np.uint8)]


def host_prep(inputs):
    """Host-side prep: feature assembly, the K=48 embedding layer h0,
    exact attention weights, demo-token phi MLP, weight packing."""
    f32 = np.float32
    times = np.asarray(inputs["times"], f32).reshape(B, T)
    values = np.asarray(inputs["values"], f32).reshape(B, T)
    meas = np.asarray(inputs["measurements"])
    demo = np.asarray(inputs["demo"], f32)
    timescales = np.asarray(inputs["timescales"], f32)
    seg_ids = np.asarray(inputs["segment_ids"])
    expect = np.repeat(np.arange(B, dtype=seg_ids.dtype), T + 1)
    assert seg_ids.shape == expect.shape and np.array_equal(seg_ids, expect), \
        "kernel assumes full-length segments (repeat(arange(B), T+1))"

    # ---- time-token features feat [B, T, 48] ----
    scaled = times[:, :, None] / timescales[None, None, :]
    feat = np.zeros((B, T, D_IN), f32)
    feat[:, :, 0:5] = np.sin(scaled)
    feat[:, :, 5:10] = np.cos(scaled)
    feat[:, :, 10] = values
    feat[:, :, 11:48] = (meas[:, :, None] ==
                         np.arange(N_MOD)[None, None, :]).astype(f32)

    # ---- demo token: encoder + full phi MLP on host (64 tokens, f32) ----
    demo_enc = np.maximum(
        demo @ np.asarray(inputs["demo_W1"], f32)
        + np.asarray(inputs["demo_b1"], f32), 0.0) \
        @ np.asarray(inputs["demo_W2"], f32) + np.asarray(inputs["demo_b2"], f32)
    h = demo_enc
    for i in range(4):
        h = np.maximum(h @ np.asarray(inputs[f"phi_W{i}"], f32)
                       + np.asarray(inputs[f"phi_b{i}"], f32), 0.0)
    enc_demo = h                                    # [B, 128]

    # ---- attention weights: e = exp(z - max) over each 4097-token segment
    W_k = np.asarray(inputs["W_k"], f32)
    W_q = np.asarray(inputs["W_q"], f32)
    M1 = np.einsum("ihd,hd->ih", W_k[:D_IN].reshape(D_IN, HEADS, DOT),
                   W_q) / np.sqrt(f32(DOT))
    z = feat @ M1                                   # [B, T, 4]
    z_demo = demo_enc @ M1                          # [B, 4]
    m = np.maximum(z.max(axis=1), z_demo)           # [B, 4]
    e_time = np.exp(z - m[:, None, :])              # [B, T, 4]
    e_demo = np.exp(z_demo - m)                     # [B, 4]
    inv = 1.0 / (e_time.sum(axis=1) + e_demo)       # [B, 4]

    # ---- embedding layer h0 = relu(W0.T x + b0) per core, [128, N] bf16
    W0T = np.ascontiguousarray(np.asarray(inputs["phi_W0"], f32).T)
    b0 = np.asarray(inputs["phi_b0"], f32)

    wpack = np.zeros((128, 256), f32)
    wpack[:, 0:128] = np.asarray(inputs["phi_W2"], f32)
    wpack[:, 128:256] = np.asarray(inputs["phi_W3"], f32)
    wpack_bf = wpack.astype(NPBF16)
    np8 = ml_dtypes.float8_e4m3
    w1 = np.asarray(inputs["phi_W1"], f32)       # [128, 128]
    wpack8 = np.ascontiguousarray(
        w1.reshape(2, 64, 128).transpose(1, 0, 2)).astype(np8) \
        .reshape(64, 256)                        # plane-major [64, 2*128]
    cpack = np.zeros((128, 3), f32)
    for i in range(3):
        cpack[:, i] = np.asarray(inputs[f"phi_b{i + 1}"], f32)
    zero_b = bool(np.all(cpack == 0.0))

    in_maps = []
    for core in range(NCORES):
        x = feat[core * SEG:(core + 1) * SEG] \
            .transpose(2, 0, 1).reshape(D_IN, N)    # [48, N]
        h0 = np.maximum(W0T @ x + b0[:, None], 0.0)  # [128, N]
        h0t = np.ascontiguousarray(
            h0.reshape(2, 64, N).transpose(1, 0, 2)).astype(np8)  # [64,2,N]
        in_maps.append({
            "h0in": h0t,
            "wpack": wpack_bf,
            "wpack8": wpack8,
            "cpack": cpack,
        })
    return in_maps, e_time, e_demo, inv, enc_demo, zero_b


def finish(enc_cores, inputs, e_time, e_demo, inv, enc_demo):
    """Attention-weighted segment sums over enc + rho MLP + sigmoid."""
    f32 = np.float32
    agg = np.empty((B, HEADS, 128), f32)
    for c in range(NCORES):
        enc_f = _fp8_to_f32(enc_cores[c])                # [128, N]
        for k in range(SEG):
            s = c * SEG + k
            a = enc_f[:, k * T:(k + 1) * T] @ e_time[s]  # [128, 4]
            a += np.outer(enc_demo[s], e_demo[s])
            agg[s] = (a * inv[s][None, :]).T
    x = agg.reshape(B, HEADS * 128)
    for i in range(3):
        x = np.maximum(x @ np.asarray(inputs[f"rho_W{i}"], f32)
                       + np.asarray(inputs[f"rho_b{i}"], f32), 0.0)
    o = x @ np.asarray(inputs["rho_W3"], f32) \
        + np.asarray(inputs["rho_b3"], f32)
    return (1.0 / (1.0 + np.exp(-o.astype(np.float64)))).astype(f32)


def kernel(**inputs):
    in_maps, e_time, e_demo, inv, enc_demo, zero_b = host_prep(inputs)
    nc = get_nc(zero_b)
    res = run_bass_kernel_spmd(nc, in_maps, core_ids=list(range(NCORES)))
    enc_cores = [res.results[c]["enc"] for c in range(NCORES)]
    return finish(enc_cores, inputs, e_time, e_demo, inv, enc_demo)


# revision 28
# speedup vs baseline: 1.0603x; 1.0603x over previous
"""Trainium2 Bass kernel for nn_DeepSetAttentionModel (segment_reduce).

Division of labour (device does the three dense 128x128 phi layers, ~90% of
the model FLOPs; host does O(N*small) prep and reductions):
  * Host assembles the 48-dim token features (sin/cos positional enc, value,
    one-hot measurement) and the first embedding layer h0 = relu(W0.T x + b0)
    (K=48, 11% of FLOPs) -- one [128,48]@[48,N] sgemm per core -- shipping
    h0 per core to HBM as fp8e4, pre-split into two 64-partition k-subtile
    planes [64, 2, N] for the PE's DoubleRow mode.
  * The psi-MLP / segment-mean branch adds a per-segment constant per head to
    the attention logits; segment softmax is invariant to it, so the whole
    psi branch cancels and is dropped.
  * Attention logits are rank-4 per token (z = x @ M1 with M1 folded from
    W_k[:48] . W_q); host computes the segment softmax weights exactly.
  * The 64 demo tokens (1 per segment) go through the phi MLP on host in
    f32; the device stream is exactly 8 segments x 4096 time tokens per
    core -- no padding columns.
  * Device: phi layer 1 runs as fp8 DoubleRow matmuls (2 output cols/cycle,
    host supplies the split operand layout for free); layers 2-3 in bf16.
    Emission is layer-major over half-core blocks (L1 over 2 super-blocks,
    then L2, then L3) so every layer's inputs are drained long before the
    PE reaches them -- the pipeline fills exactly once.  PSUM->SBUF relu
    drains alternate between the Activation and Vector engines on 2-bank
    (1024-col) groups, 4 in flight, slightly ACT-biased to match the
    engines' 1.2 vs 0.96 GHz clocks; this pair of engines is the throughput
    floor of the kernel.  enc [128, 32768] fp8e4 streams back to HBM in
    2048-col pieces, overlapped with compute.  All DMA runs on SP's HWDGE
    queue in consumption order (one queue moves ~330GB/s; the 16 DMA
    engines are shared, so a second queue only steals from the head), as
    2D-contiguous per-plane slices -- 3D strided pieces are ~10x slower.
  * Host: attention-weighted segment sums over enc (O(N*512) f32 BLAS),
    then the tiny rho MLP 512->128->128->128->1 + sigmoid.

Sharding: data-parallel across patients -- 8 whole segments per core,
weights replicated, no collectives.
"""

import numpy as np
import ml_dtypes

import concourse.tile as tile
from concourse import bacc, mybir
from concourse.bass_utils import run_bass_kernel_spmd

F32 = mybir.dt.float32
BF16 = mybir.dt.bfloat16
FP8 = mybir.dt.float8e4
AF = mybir.ActivationFunctionType
ALU = mybir.AluOpType
NPBF16 = ml_dtypes.bfloat16

NCORES = 8
B, T = 64, 4096
SEG = 8                  # segments per core
N = SEG * T              # 32768 device tokens per core
D_IN = 48
HEADS, DOT = 4, 64
N_MOD = 37

SB_SEGS = 2              # segments per super-block
NSB = SEG // SB_SEGS     # 4 super-blocks
SBW = SB_SEGS * T        # 8192 cols per super-block
CH = 512                 # matmul moving width (one PSUM bank of f32)
GRP = 2 * CH             # drain group: 2 banks = 1024 cols
NGRP = SBW // GRP        # 8 drain groups per layer per super-block
ENC_DMA = 2 * GRP        # enc DMA piece: 2048 cols (512KB)

_CACHE = {}


def _build(zero_b):
    nc = bacc.Bacc(
        "TRN2",
        target_bir_lowering=False,
        debug=False,
        enable_asserts=False,
        num_devices=NCORES,
    )

    io = {
        "h0in": nc.dram_tensor("h0in", [64, 2, N], FP8,
                               kind="ExternalInput").ap(),
        "wpack": nc.dram_tensor("wpack", [128, 256], BF16,
                                kind="ExternalInput").ap(),
        "wpack8": nc.dram_tensor("wpack8", [64, 256], FP8,
                                 kind="ExternalInput").ap(),
        "cpack": nc.dram_tensor("cpack", [128, 3], F32,
                                kind="ExternalInput").ap(),
        "enc": nc.dram_tensor("enc", [128, N], FP8, kind="ExternalOutput").ap(),
    }

    with tile.TileContext(nc) as tc:
        _emit(tc, io, zero_b)

    _dedup_ldweights(nc)
    nc.compile()
    return nc


def _ldw_key(inst):
    ap = inst.ins[0]
    return (
        getattr(ap, "memref", None),
        ap.offset,
        tuple(tuple(p) for p in ap.ap),
        str(ap.dtype),
        str(getattr(inst, "tile_position", None)),
        str(getattr(inst, "perf_mode", None)),
        bool(inst.is_transpose or False),
    )


def _dedup_ldweights(nc):
    """Drop InstLdweights that reload the stationary operand already in the
    PE array (identical weights AP, no intervening PE weight writes).  The
    PE keeps weights across matmuls, so the reload is semantically a no-op
    but costs ~90ns and breaks back-to-back matmul fill/drain pipelining.
    Dropped instructions' semaphore waits transfer to the next PE
    instruction."""
    removed = 0
    for fn in nc.m.functions:
        for b in fn.blocks:
            last_key = None
            pending_waits = []
            keep = []
            for inst in b.instructions:
                eng = getattr(inst, "engine", None)
                if isinstance(inst, mybir.InstLdweights):
                    key = _ldw_key(inst)
                    si = inst.sync_info
                    if key == last_key and not (si and si.on_update):
                        if si and si.on_wait:
                            pending_waits.extend(si.on_wait)
                        removed += 1
                        continue
                    last_key = key
                elif isinstance(inst, mybir.InstMatmult):
                    if inst.ldweights:
                        last_key = None
                elif eng == mybir.EngineType.PE and not inst.is_sequencer_only():
                    last_key = None
                if pending_waits and eng == mybir.EngineType.PE:
                    si = inst.sync_info
                    if si is None:
                        inst.sync_info = mybir.SyncInfo(
                            on_wait=list(pending_waits), on_update=[])
                    else:
                        si.on_wait = list(si.on_wait) + pending_waits
                    pending_waits = []
                keep.append(inst)
            assert not pending_waits, "dropped LDW waits with no PE successor"
            b.instructions[:] = keep
    return removed


def _emit(tc, io, zero_b):
    nc = tc.nc
    sync = nc.sync
    act = nc.scalar
    dve = nc.vector
    pe = nc.tensor

    with tc.tile_pool(name="const", bufs=1) as cp:
        wsb8 = cp.tile([64, 2, 128], FP8, tag="wsb8")
        for p8 in range(2):
            sync.dma_start(wsb8[:, p8, :], io["wpack8"][:, p8 * 128:(p8 + 1) * 128])

        # h0 super-block tiles.  One HWDGE queue moves ~330GB/s and each
        # dma_start costs SP ~600ns to issue, so pieces are per-plane 2D
        # slices (3D strided DMA runs ~10x slower) sized to keep arrival
        # ahead of the PE with few issues: sb0 in 256KB halves (both planes
        # of the first 4096 cols land before the first matmul), the rest as
        # whole-plane 512KB transfers, all in consumption order.
        hb = [cp.tile([64, 2, SBW], FP8, tag=f"h0_{sb}", name=f"h0_{sb}")
              for sb in range(NSB)]
        head = 2048
        for p in range(2):
            sync.dma_start(hb[0][:, p, 0:head], io["h0in"][:, p, 0:head])
        for p in range(2):
            sync.dma_start(hb[0][:, p, head:SBW],
                           io["h0in"][:, p, head:SBW])
        wsb = cp.tile([128, 256], BF16, tag="wsb")
        sync.dma_start(wsb, io["wpack"])
        csb = cp.tile([128, 3], F32, tag="csb")
        sync.dma_start(csb, io["cpack"])
        for sb in range(1, NSB):
            for p in range(2):
                sync.dma_start(hb[sb][:, p, :],
                               io["h0in"][:, p, sb * SBW:(sb + 1) * SBW])

        w = [wsb8, wsb[:, 0:128], wsb[:, 128:256]]
        pb = [csb[:, i:i + 1] for i in range(3)]

        # Warm each drain engine during the DMA prologue: the first ACT use
        # pays a ~1.3us activation-table load, the first DVE op pays a
        # similar tensor load.
        wa = cp.tile([1, 1], F32, tag="warma")
        dve.memset(wa, 0.0)
        act.activation(wa, wa, AF.Relu)
        wv = cp.tile([1, 1], F32, tag="warmv")
        dve.memset(wv, 0.0)
        dve.tensor_scalar(wv, wv, 0.0, 0.0, ALU.add, ALU.max)

        # Half-core layer-major schedule: L1 over two super-blocks, then L2,
        # then L3.  Every layer's inputs are fully drained long before the
        # PE reaches them, so there are no layer-transition bubbles -- the
        # pipeline fills exactly once, at the very start.
        with tc.tile_pool(name="hbuf", bufs=2) as hp, \
             tc.tile_pool(name="encb", bufs=2) as ep, \
             tc.tile_pool(name="ps", bufs=4, space="PSUM") as pp:
            for half in range(NSB // 2):
                sbs = (2 * half, 2 * half + 1)
                h1 = {s: hp.tile([128, SBW], BF16, tag="h1", name=f"h1_{s}")
                      for s in sbs}
                h2 = {s: hp.tile([128, SBW], BF16, tag="h2", name=f"h2_{s}")
                      for s in sbs}
                enc = {s: ep.tile([128, SBW], FP8, tag="enc", name=f"enc_{s}")
                       for s in sbs}
                for li in range(3):
                    for sb in sbs:
                        src = (hb[sb], h1[sb], h2[sb])[li]
                        dst = (h1[sb], h2[sb], enc[sb])[li]
                        wk, bk = w[li], pb[li]
                        pat = "ADADADAD"
                        for g in range(NGRP):
                            ps = pp.tile([128, GRP], F32, tag="ps",
                                         name=f"ps{sb}_{li}_{g}")
                            for c in range(GRP // CH):
                                col = g * GRP + c * CH
                                if li == 0:
                                    # fp8 DoubleRow: K=128 as two 64-row
                                    # subtiles; host ships h0/w1 pre-split.
                                    pe.matmul(ps[:, c * CH:(c + 1) * CH], wk,
                                              src[:, :, col:col + CH],
                                              start=True, stop=True,
                                              perf_mode=mybir.MatmulPerfMode
                                              .DoubleRow)
                                else:
                                    pe.matmul(ps[:, c * CH:(c + 1) * CH], wk,
                                              src[:, col:col + CH],
                                              start=True, stop=True)
                            dv = dst[:, g * GRP:(g + 1) * GRP]
                            if pat[g] == "A":
                                if zero_b:
                                    act.activation(dv, ps, AF.Relu)
                                else:
                                    act.activation(dv, ps, AF.Relu, bias=bk)
                            elif zero_b:
                                dve.tensor_scalar_max(dv, ps, 0.0)
                            else:
                                dve.tensor_scalar(dv, ps, bk, 0.0,
                                                  ALU.add, ALU.max)
                            piece = ENC_DMA
                            if li == 2 and (g + 1) % (piece // GRP) == 0:
                                base = (g + 1) * GRP - piece
                                seg0 = sb * SB_SEGS
                                sync.dma_start(
                                    io["enc"][:, seg0 * T + base:
                                              seg0 * T + base + piece],
                                    dst[:, base:base + piece])


def get_nc(zero_b):
    key = ("nc", zero_b)
    if key not in _CACHE:
        _CACHE[key] = _build(zero_b)
    return _CACHE[key]


def _fp8_to_f32(a):
    lut = _CACHE.get("fp8lut")
    if lut is None:
        lut = np.arange(256, dtype=np.uint8).view(ml_dtypes.float8_e4m3) \
            .astype(np.float32)
        _CACHE["fp8lut"] = lut
    return lut[np.asarray(a).view(np.uint8)]


def host_prep(inputs):
    """Host-side prep: feature assembly, the K=48 embedding layer h0,
    exact attention weights, demo-token phi MLP, weight packing."""
    f32 = np.float32
    times = np.asarray(inputs["times"], f32).reshape(B, T)
    values = np.asarray(inputs["values"], f32).reshape(B, T)
    meas = np.asarray(inputs["measurements"])
    demo = np.asarray(inputs["demo"], f32)
    timescales = np.asarray(inputs["timescales"], f32)
    seg_ids = np.asarray(inputs["segment_ids"])
    expect = np.repeat(np.arange(B, dtype=seg_ids.dtype), T + 1)
    assert seg_ids.shape == expect.shape and np.array_equal(seg_ids, expect), \
        "kernel assumes full-length segments (repeat(arange(B), T+1))"

    # ---- time-token features feat [B, T, 48] ----
    scaled = times[:, :, None] / timescales[None, None, :]
    feat = np.zeros((B, T, D_IN), f32)
    feat[:, :, 0:5] = np.sin(scaled)
    feat[:, :, 5:10] = np.cos(scaled)
    feat[:, :, 10] = values
    feat[:, :, 11:48] = (meas[:, :, None] ==
                         np.arange(N_MOD)[None, None, :]).astype(f32)

    # ---- demo token: encoder + full phi MLP on host (64 tokens, f32) ----
    demo_enc = np.maximum(
        demo @ np.asarray(inputs["demo_W1"], f32)
        + np.asarray(inputs["demo_b1"], f32), 0.0) \
        @ np.asarray(inputs["demo_W2"], f32) + np.asarray(inputs["demo_b2"], f32)
    h = demo_enc
    for i in range(4):
        h = np.maximum(h @ np.asarray(inputs[f"phi_W{i}"], f32)
                       + np.asarray(inputs[f"phi_b{i}"], f32), 0.0)
    enc_demo = h                                    # [B, 128]

    # ---- attention weights: e = exp(z - max) over each 4097-token segment
    W_k = np.asarray(inputs["W_k"], f32)
    W_q = np.asarray(inputs["W_q"], f32)
    M1 = np.einsum("ihd,hd->ih", W_k[:D_IN].reshape(D_IN, HEADS, DOT),
                   W_q) / np.sqrt(f32(DOT))
    z = feat @ M1                                   # [B, T, 4]
    z_demo = demo_enc @ M1                          # [B, 4]
    m = np.maximum(z.max(axis=1), z_demo)           # [B, 4]
    e_time = np.exp(z - m[:, None, :])              # [B, T, 4]
    e_demo = np.exp(z_demo - m)                     # [B, 4]
    inv = 1.0 / (e_time.sum(axis=1) + e_demo)       # [B, 4]

    # ---- embedding layer h0 = relu(W0.T x + b0) per core, [128, N] bf16
    W0T = np.ascontiguousarray(np.asarray(inputs["phi_W0"], f32).T)
    b0 = np.asarray(inputs["phi_b0"], f32)

    wpack = np.zeros((128, 256), f32)
    wpack[:, 0:128] = np.asarray(inputs["phi_W2"], f32)
    wpack[:, 128:256] = np.asarray(inputs["phi_W3"], f32)
    wpack_bf = wpack.astype(NPBF16)
    np8 = ml_dtypes.float8_e4m3
    w1 = np.asarray(inputs["phi_W1"], f32)       # [128, 128]
    wpack8 = np.ascontiguousarray(
        w1.reshape(2, 64, 128).transpose(1, 0, 2)).astype(np8) \
        .reshape(64, 256)                        # plane-major [64, 2*128]
    cpack = np.zeros((128, 3), f32)
    for i in range(3):
        cpack[:, i] = np.asarray(inputs[f"phi_b{i + 1}"], f32)
    zero_b = bool(np.all(cpack == 0.0))

    in_maps = []
    for core in range(NCORES):
        x = feat[core * SEG:(core + 1) * SEG] \
            .transpose(2, 0, 1).reshape(D_IN, N)    # [48, N]
        h0 = np.maximum(W0T @ x + b0[:, None], 0.0)  # [128, N]
        h0t = np.ascontiguousarray(
            h0.reshape(2, 64, N).transpose(1, 0, 2)).astype(np8)  # [64,2,N]
        in_maps.append({
            "h0in": h0t,
            "wpack": wpack_bf,
            "wpack8": wpack8,
            "cpack": cpack,
        })
    return in_maps, e_time, e_demo, inv, enc_demo, zero_b


def finish(enc_cores, inputs, e_time, e_demo, inv, enc_demo):
    """Attention-weighted segment sums over enc + rho MLP + sigmoid."""
    f32 = np.float32
    agg = np.empty((B, HEADS, 128), f32)
    for c in range(NCORES):
        enc_f = _fp8_to_f32(enc_cores[c])                # [128, N]
        for k in range(SEG):
            s = c * SEG + k
            a = enc_f[:, k * T:(k + 1) * T] @ e_time[s]  # [128, 4]
            a += np.outer(enc_demo[s], e_demo[s])
            agg[s] = (a * inv[s][None, :]).T
    x = agg.reshape(B, HEADS * 128)
    for i in range(3):
        x = np.maximum(x @ np.asarray(inputs[f"rho_W{i}"], f32)
                       + np.asarray(inputs[f"rho_b{i}"], f32), 0.0)
    o = x @ np.asarray(inputs["rho_W3"], f32) \
        + np.asarray(inputs["rho_b3"], f32)
    return (1.0 / (1.0 + np.exp(-o.astype(np.float64)))).astype(f32)


def kernel(**inputs):
    in_maps, e_time, e_demo, inv, enc_demo, zero_b = host_prep(inputs)
    nc = get_nc(zero_b)
    res = run_bass_kernel_spmd(nc, in_maps, core_ids=list(range(NCORES)))
    enc_cores = [res.results[c]["enc"] for c in range(NCORES)]
    return finish(enc_cores, inputs, e_time, e_demo, inv, enc_demo)


# revision 29
# speedup vs baseline: 1.0798x; 1.0184x over previous
"""Trainium2 Bass kernel for nn_DeepSetAttentionModel (segment_reduce).

Division of labour (device does the three dense 128x128 phi layers, ~90% of
the model FLOPs; host does O(N*small) prep and reductions):
  * Host assembles the 48-dim token features (sin/cos positional enc, value,
    one-hot measurement) and the first embedding layer h0 = relu(W0.T x + b0)
    (K=48, 11% of FLOPs) -- one [128,48]@[48,N] sgemm per core -- shipping
    h0 per core to HBM as fp8e4, pre-split into two 64-partition k-subtile
    planes [64, 2, N] for the PE's DoubleRow mode.
  * The psi-MLP / segment-mean branch adds a per-segment constant per head to
    the attention logits; segment softmax is invariant to it, so the whole
    psi branch cancels and is dropped.
  * Attention logits are rank-4 per token (z = x @ M1 with M1 folded from
    W_k[:48] . W_q); host computes the segment softmax weights exactly.
  * The 64 demo tokens (1 per segment) go through the phi MLP on host in
    f32; the device stream is exactly 8 segments x 4096 time tokens per
    core -- no padding columns.
  * Device: phi layer 1 runs as fp8 DoubleRow matmuls (2 output cols/cycle,
    host supplies the split operand layout for free); layers 2-3 in bf16.
    Emission is layer-major over half-core blocks (L1 over 2 super-blocks,
    then L2, then L3) so every layer's inputs are drained long before the
    PE reaches them -- the pipeline fills exactly once.  PSUM->SBUF relu
    drains alternate between the Activation and Vector engines on 2-bank
    (1024-col) groups, 4 in flight; this pair of engines is the throughput
    floor of the kernel.  enc [128, 32768] fp8e4 streams back to HBM in
    2048-col pieces, overlapped with compute.  All DMA runs on SP's HWDGE
    queue in consumption order (one queue moves ~330GB/s; the 16 DMA
    engines are shared, so a second queue only steals from the head), as
    2D-contiguous per-plane slices -- 3D strided pieces are ~10x slower.
  * Host: attention-weighted segment sums over enc (O(N*512) f32 BLAS),
    then the tiny rho MLP 512->128->128->128->1 + sigmoid.

Sharding: data-parallel across patients -- 8 whole segments per core,
weights replicated, no collectives.
"""

import numpy as np
import ml_dtypes

import concourse.tile as tile
from concourse import bacc, mybir
from concourse.bass_utils import run_bass_kernel_spmd

F32 = mybir.dt.float32
BF16 = mybir.dt.bfloat16
FP8 = mybir.dt.float8e4
AF = mybir.ActivationFunctionType
ALU = mybir.AluOpType
NPBF16 = ml_dtypes.bfloat16

NCORES = 8
B, T = 64, 4096
SEG = 8                  # segments per core
N = SEG * T              # 32768 device tokens per core
D_IN = 48
HEADS, DOT = 4, 64
N_MOD = 37

SB_SEGS = 2              # segments per super-block
NSB = SEG // SB_SEGS     # 4 super-blocks
SBW = SB_SEGS * T        # 8192 cols per super-block
CH = 512                 # matmul moving width (one PSUM bank of f32)
GRP = 2 * CH             # drain group: 2 banks = 1024 cols
NGRP = SBW // GRP        # 8 drain groups per layer per super-block
ENC_DMA = 2 * GRP        # enc DMA piece: 2048 cols (512KB)

_CACHE = {}


def _build(zero_b):
    nc = bacc.Bacc(
        "TRN2",
        target_bir_lowering=False,
        debug=False,
        enable_asserts=False,
        num_devices=NCORES,
    )

    io = {
        "h0in": nc.dram_tensor("h0in", [64, 2, N], FP8,
                               kind="ExternalInput").ap(),
        "wpack": nc.dram_tensor("wpack", [128, 256], BF16,
                                kind="ExternalInput").ap(),
        "wpack8": nc.dram_tensor("wpack8", [64, 256], FP8,
                                 kind="ExternalInput").ap(),
        "cpack": nc.dram_tensor("cpack", [128, 3], F32,
                                kind="ExternalInput").ap(),
        "enc": nc.dram_tensor("enc", [128, N], FP8, kind="ExternalOutput").ap(),
    }

    with tile.TileContext(nc) as tc:
        _emit(tc, io, zero_b)

    _dedup_ldweights(nc)
    nc.compile()
    return nc


def _ldw_key(inst):
    ap = inst.ins[0]
    return (
        getattr(ap, "memref", None),
        ap.offset,
        tuple(tuple(p) for p in ap.ap),
        str(ap.dtype),
        str(getattr(inst, "tile_position", None)),
        str(getattr(inst, "perf_mode", None)),
        bool(inst.is_transpose or False),
    )


def _dedup_ldweights(nc):
    """Drop InstLdweights that reload the stationary operand already in the
    PE array (identical weights AP, no intervening PE weight writes).  The
    PE keeps weights across matmuls, so the reload is semantically a no-op
    but costs ~90ns and breaks back-to-back matmul fill/drain pipelining.
    Dropped instructions' semaphore waits transfer to the next PE
    instruction."""
    removed = 0
    for fn in nc.m.functions:
        for b in fn.blocks:
            last_key = None
            pending_waits = []
            keep = []
            for inst in b.instructions:
                eng = getattr(inst, "engine", None)
                if isinstance(inst, mybir.InstLdweights):
                    key = _ldw_key(inst)
                    si = inst.sync_info
                    if key == last_key and not (si and si.on_update):
                        if si and si.on_wait:
                            pending_waits.extend(si.on_wait)
                        removed += 1
                        continue
                    last_key = key
                elif isinstance(inst, mybir.InstMatmult):
                    if inst.ldweights:
                        last_key = None
                elif eng == mybir.EngineType.PE and not inst.is_sequencer_only():
                    last_key = None
                if pending_waits and eng == mybir.EngineType.PE:
                    si = inst.sync_info
                    if si is None:
                        inst.sync_info = mybir.SyncInfo(
                            on_wait=list(pending_waits), on_update=[])
                    else:
                        si.on_wait = list(si.on_wait) + pending_waits
                    pending_waits = []
                keep.append(inst)
            assert not pending_waits, "dropped LDW waits with no PE successor"
            b.instructions[:] = keep
    return removed


def _emit(tc, io, zero_b):
    nc = tc.nc
    sync = nc.sync
    act = nc.scalar
    dve = nc.vector
    pe = nc.tensor

    with tc.tile_pool(name="const", bufs=1) as cp:
        wsb8 = cp.tile([64, 2, 128], FP8, tag="wsb8")
        for p8 in range(2):
            sync.dma_start(wsb8[:, p8, :], io["wpack8"][:, p8 * 128:(p8 + 1) * 128])

        # h0 super-block tiles.  One HWDGE queue moves ~330GB/s and each
        # dma_start costs SP ~600ns to issue, so pieces are per-plane 2D
        # slices (3D strided DMA runs ~10x slower) sized to keep arrival
        # ahead of the PE with few issues: sb0 in 256KB halves (both planes
        # of the first 4096 cols land before the first matmul), the rest as
        # whole-plane 512KB transfers, all in consumption order.
        hb = [cp.tile([64, 2, SBW], FP8, tag=f"h0_{sb}", name=f"h0_{sb}")
              for sb in range(NSB)]
        head = 2048
        for p in range(2):
            sync.dma_start(hb[0][:, p, 0:head], io["h0in"][:, p, 0:head])
        for p in range(2):
            sync.dma_start(hb[0][:, p, head:SBW],
                           io["h0in"][:, p, head:SBW])
        wsb = cp.tile([128, 256], BF16, tag="wsb")
        sync.dma_start(wsb, io["wpack"])
        csb = cp.tile([128, 3], F32, tag="csb")
        sync.dma_start(csb, io["cpack"])
        for sb in range(1, NSB):
            for p in range(2):
                sync.dma_start(hb[sb][:, p, :],
                               io["h0in"][:, p, sb * SBW:(sb + 1) * SBW])

        w = [wsb8, wsb[:, 0:128], wsb[:, 128:256]]
        pb = [csb[:, i:i + 1] for i in range(3)]

        # Warm each drain engine during the DMA prologue: the first ACT use
        # pays a ~1.3us activation-table load, the first DVE op pays a
        # similar tensor load.
        wa = cp.tile([1, 1], F32, tag="warma")
        dve.memset(wa, 0.0)
        act.activation(wa, wa, AF.Relu)
        wv = cp.tile([1, 1], F32, tag="warmv")
        dve.memset(wv, 0.0)
        dve.tensor_scalar(wv, wv, 0.0, 0.0, ALU.add, ALU.max)

        # Half-core layer-major schedule: L1 over two super-blocks, then L2,
        # then L3.  Every layer's inputs are fully drained long before the
        # PE reaches them, so there are no layer-transition bubbles -- the
        # pipeline fills exactly once, at the very start.
        with tc.tile_pool(name="hbuf", bufs=2) as hp, \
             tc.tile_pool(name="encb", bufs=2) as ep, \
             tc.tile_pool(name="ps", bufs=4, space="PSUM") as pp:
            for half in range(NSB // 2):
                sbs = (2 * half, 2 * half + 1)
                h1 = {s: hp.tile([128, SBW], BF16, tag="h1", name=f"h1_{s}")
                      for s in sbs}
                h2 = {s: hp.tile([128, SBW], BF16, tag="h2", name=f"h2_{s}")
                      for s in sbs}
                enc = {s: ep.tile([128, SBW], FP8, tag="enc", name=f"enc_{s}")
                       for s in sbs}
                for li in range(3):
                    for sb in sbs:
                        src = (hb[sb], h1[sb], h2[sb])[li]
                        dst = (h1[sb], h2[sb], enc[sb])[li]
                        wk, bk = w[li], pb[li]
                        pat = "ADADADAD"
                        for g in range(NGRP):
                            ps = pp.tile([128, GRP], F32, tag="ps",
                                         name=f"ps{sb}_{li}_{g}")
                            for c in range(GRP // CH):
                                col = g * GRP + c * CH
                                if li == 0:
                                    # fp8 DoubleRow: K=128 as two 64-row
                                    # subtiles; host ships h0/w1 pre-split.
                                    pe.matmul(ps[:, c * CH:(c + 1) * CH], wk,
                                              src[:, :, col:col + CH],
                                              start=True, stop=True,
                                              perf_mode=mybir.MatmulPerfMode
                                              .DoubleRow)
                                else:
                                    pe.matmul(ps[:, c * CH:(c + 1) * CH], wk,
                                              src[:, col:col + CH],
                                              start=True, stop=True)
                            dv = dst[:, g * GRP:(g + 1) * GRP]
                            if pat[g] == "A":
                                if zero_b:
                                    act.activation(dv, ps, AF.Relu)
                                else:
                                    act.activation(dv, ps, AF.Relu, bias=bk)
                            elif zero_b:
                                dve.tensor_scalar_max(dv, ps, 0.0)
                            else:
                                dve.tensor_scalar(dv, ps, bk, 0.0,
                                                  ALU.add, ALU.max)
                            piece = ENC_DMA
                            if li == 2 and (g + 1) % (piece // GRP) == 0:
                                base = (g + 1) * GRP - piece
                                seg0 = sb * SB_SEGS
                                sync.dma_start(
                                    io["enc"][:, seg0 * T + base:
                                              seg0 * T + base + piece],
                                    dst[:, base:base + piece])


def get_nc(zero_b):
    key = ("nc", zero_b)
    if key not in _CACHE:
        _CACHE[key] = _build(zero_b)
    return _CACHE[key]


def _fp8_to_f32(a):
    lut = _CACHE.get("fp8lut")
    if lut is None:
        lut = np.arange(256, dtype=np.uint8).view(ml_dtypes.float8_e4m3) \
            .astype(np.float32)
        _CACHE["fp8lut"] = lut
    return lut[np.asarray(a).view(np.uint8)]


def host_prep(inputs):
    """Host-side prep: feature assembly, the K=48 embedding layer h0,
    exact attention weights, demo-token phi MLP, weight packing."""
    f32 = np.float32
    times = np.asarray(inputs["times"], f32).reshape(B, T)
    values = np.asarray(inputs["values"], f32).reshape(B, T)
    meas = np.asarray(inputs["measurements"])
    demo = np.asarray(inputs["demo"], f32)
    timescales = np.asarray(inputs["timescales"], f32)
    seg_ids = np.asarray(inputs["segment_ids"])
    expect = np.repeat(np.arange(B, dtype=seg_ids.dtype), T + 1)
    assert seg_ids.shape == expect.shape and np.array_equal(seg_ids, expect), \
        "kernel assumes full-length segments (repeat(arange(B), T+1))"

    # ---- time-token features feat [B, T, 48] ----
    scaled = times[:, :, None] / timescales[None, None, :]
    feat = np.zeros((B, T, D_IN), f32)
    feat[:, :, 0:5] = np.sin(scaled)
    feat[:, :, 5:10] = np.cos(scaled)
    feat[:, :, 10] = values
    feat[:, :, 11:48] = (meas[:, :, None] ==
                         np.arange(N_MOD)[None, None, :]).astype(f32)

    # ---- demo token: encoder + full phi MLP on host (64 tokens, f32) ----
    demo_enc = np.maximum(
        demo @ np.asarray(inputs["demo_W1"], f32)
        + np.asarray(inputs["demo_b1"], f32), 0.0) \
        @ np.asarray(inputs["demo_W2"], f32) + np.asarray(inputs["demo_b2"], f32)
    h = demo_enc
    for i in range(4):
        h = np.maximum(h @ np.asarray(inputs[f"phi_W{i}"], f32)
                       + np.asarray(inputs[f"phi_b{i}"], f32), 0.0)
    enc_demo = h                                    # [B, 128]

    # ---- attention weights: e = exp(z - max) over each 4097-token segment
    W_k = np.asarray(inputs["W_k"], f32)
    W_q = np.asarray(inputs["W_q"], f32)
    M1 = np.einsum("ihd,hd->ih", W_k[:D_IN].reshape(D_IN, HEADS, DOT),
                   W_q) / np.sqrt(f32(DOT))
    z = feat @ M1                                   # [B, T, 4]
    z_demo = demo_enc @ M1                          # [B, 4]
    m = np.maximum(z.max(axis=1), z_demo)           # [B, 4]
    e_time = np.exp(z - m[:, None, :])              # [B, T, 4]
    e_demo = np.exp(z_demo - m)                     # [B, 4]
    inv = 1.0 / (e_time.sum(axis=1) + e_demo)       # [B, 4]

    # ---- embedding layer h0 = relu(W0.T x + b0) per core (fp8, split)
    W0T = np.ascontiguousarray(np.asarray(inputs["phi_W0"], f32).T)
    b0 = np.asarray(inputs["phi_b0"], f32)

    wpack = np.zeros((128, 256), f32)
    wpack[:, 0:128] = np.asarray(inputs["phi_W2"], f32)
    wpack[:, 128:256] = np.asarray(inputs["phi_W3"], f32)
    wpack_bf = wpack.astype(NPBF16)
    np8 = ml_dtypes.float8_e4m3
    w1 = np.asarray(inputs["phi_W1"], f32)       # [128, 128]
    wpack8 = np.ascontiguousarray(
        w1.reshape(2, 64, 128).transpose(1, 0, 2)).astype(np8) \
        .reshape(64, 256)                        # plane-major [64, 2*128]
    cpack = np.zeros((128, 3), f32)
    for i in range(3):
        cpack[:, i] = np.asarray(inputs[f"phi_b{i + 1}"], f32)
    zero_b = bool(np.all(cpack == 0.0))

    in_maps = []
    for core in range(NCORES):
        x = feat[core * SEG:(core + 1) * SEG] \
            .transpose(2, 0, 1).reshape(D_IN, N)    # [48, N]
        h0 = np.maximum(W0T @ x + b0[:, None], 0.0)  # [128, N]
        h0t = np.ascontiguousarray(
            h0.reshape(2, 64, N).transpose(1, 0, 2)).astype(np8)  # [64,2,N]
        in_maps.append({
            "h0in": h0t,
            "wpack": wpack_bf,
            "wpack8": wpack8,
            "cpack": cpack,
        })
    return in_maps, e_time, e_demo, inv, enc_demo, zero_b


def finish(enc_cores, inputs, e_time, e_demo, inv, enc_demo):
    """Attention-weighted segment sums over enc + rho MLP + sigmoid."""
    f32 = np.float32
    agg = np.empty((B, HEADS, 128), f32)
    for c in range(NCORES):
        enc_f = _fp8_to_f32(enc_cores[c])                # [128, N]
        for k in range(SEG):
            s = c * SEG + k
            a = enc_f[:, k * T:(k + 1) * T] @ e_time[s]  # [128, 4]
            a += np.outer(enc_demo[s], e_demo[s])
            agg[s] = (a * inv[s][None, :]).T
    x = agg.reshape(B, HEADS * 128)
    for i in range(3):
        x = np.maximum(x @ np.asarray(inputs[f"rho_W{i}"], f32)
                       + np.asarray(inputs[f"rho_b{i}"], f32), 0.0)
    o = x @ np.asarray(inputs["rho_W3"], f32) \
        + np.asarray(inputs["rho_b3"], f32)
    return (1.0 / (1.0 + np.exp(-o.astype(np.float64)))).astype(f32)


def kernel(**inputs):
    in_maps, e_time, e_demo, inv, enc_demo, zero_b = host_prep(inputs)
    nc = get_nc(zero_b)
    res = run_bass_kernel_spmd(nc, in_maps, core_ids=list(range(NCORES)))
    enc_cores = [res.results[c]["enc"] for c in range(NCORES)]
    return finish(enc_cores, inputs, e_time, e_demo, inv, enc_demo)
